# revision 11
# baseline (speedup 1.0000x reference)
"""Chamfer loss kernel v5 for Trainium2, 8 NeuronCores.

Math: T[i,j] = -||g_i - p_j||^2 / 2 computed in PSUM by K=13 bf16
matmuls using an exact hi/lo split (see make_blocks).

v5 = v2's DVE/ACT schedule + 4-way PE row-group packing:
  - K=13 fits in one 32-row group, so FOUR matmuls run concurrently at
    tile_position (32i, 0), i=0..3. Both operands are replicated at
    partition offsets {0,32,64,96} host-side.
  - Each PSUM group [128,2048] = ONE gt row-tile x FOUR adjacent
    512-wide pred windows (segment i = window 4*cq+i). This keeps the
    staged tile homogeneous in the row direction, so the v2-style
    big-FD DVE ops still apply:
      ACT: one [128,2048] PSUM->SBUF bf16 copy per group (~2us)
      row: one tensor_scalar accum-max FD=2048 (4x mode, fp8 junk out)
      col: one in-place tensor_tensor max FD=2048 into colmax
    First tile per window-quad uses v2's fused op (ts main-out writes
    colmax init + accum row max).
  - Col tail: gpsimd partition_all_reduce(max) + DVE accum-add.
  - Packed MMs cut PE busy from 214us to ~96us/core (hidden under DVE).

loss = -2 * (sum_i max_j T + sum_j max_i T) / (B*N), summed on host.

Sharding: batch dim 16 -> 2 per core.
"""

import sys

import numpy as np

sys.path.insert(0, "/opt/trn_rl_repo")

import ml_dtypes  # noqa: E402

import concourse.bass as bass  # noqa: E402
import concourse.mybir as mybir  # noqa: E402
import concourse.tile as tile  # noqa: E402
from concourse import bacc  # noqa: E402
from concourse.bass_utils import run_bass_kernel_spmd  # noqa: E402

BF16 = ml_dtypes.bfloat16

B, N_FULL, D = 16, 4096, 3
NCORES = 8
BLOC = B // NCORES  # batches per core
KR = 13  # matmul contraction rows
SEG = 512  # psum bank / matmul moving width
QUAD = 4  # windows packed per psum group (one per PE row group)

_built = {}


def build(n=N_FULL, bloc=BLOC, reps=1, probe="full", junk8=True, gcol=0):
    """Per-core Bass module. Returns (nc, "Lblk", "Rblk", "out")."""
    key = (n, bloc, reps, probe, junk8)
    if key in _built:
        return _built[key]
    do_act = probe != "mm"
    do_row = probe in ("full", "row")
    do_col = probe in ("full", "col")

    nt = n // 128  # row tiles (32)
    nc_ = n // SEG  # pred windows (8)
    ncq = nc_ // QUAD  # window quads (2)
    group = QUAD * SEG  # staged tile width (2048)

    nc = bacc.Bacc("TRN2", target_bir_lowering=False, debug=False)
    from concourse import bass_isa
    bf = mybir.dt.bfloat16
    fp = mybir.dt.float32
    jdt = mybir.dt.float8e4 if junk8 else bf

    L_d = nc.dram_tensor("Lblk", [bloc, 128, n], bf, kind="ExternalInput")
    R_d = nc.dram_tensor("Rblk", [bloc, 128, n], bf, kind="ExternalInput")
    out_d = nc.dram_tensor("out", [128, 2 * bloc], fp, kind="ExternalOutput")

    with tile.TileContext(nc) as tc:
        with (
            tc.tile_pool(name="blocks", bufs=1) as blocks,
            tc.tile_pool(name="stg", bufs=4) as stg,
            tc.tile_pool(name="jnk", bufs=2) as jnk,
            tc.tile_pool(name="colp", bufs=2) as colp,
            tc.tile_pool(name="carp", bufs=2) as carp,
            tc.tile_pool(name="rowp", bufs=2) as rowp,
            tc.tile_pool(name="small", bufs=1) as small,
            tc.tile_pool(name="psum", bufs=2, space="PSUM") as psum_pool,
        ):
            out_sb = small.tile([128, 2 * bloc], fp, tag="out_sb")
            nc.scalar.memzero(out_sb[:])

            lbs, rbs = [], []
            for b in range(bloc):
                lb = blocks.tile([128, n], bf, tag=f"lb{b}")
                rb = blocks.tile([128, n], bf, tag=f"rb{b}")
                nc.sync.dma_start(out=lb[:], in_=L_d[b])
                nc.sync.dma_start(out=rb[:], in_=R_d[b])
                lbs.append(lb)
                rbs.append(rb)

            def emit_main():
                for b in range(bloc):
                    lb, rb = lbs[b], rbs[b]
                    colmax = colp.tile([128, n], bf, tag="colmax")
                    rowpart = rowp.tile([128, nt * ncq], fp, tag="rowpart")
                    for t in range(nt):
                        for cq in range(ncq):
                            ps = psum_pool.tile([128, group], fp, tag="ps")
                            for i in range(QUAD):
                                w = (QUAD * cq + i) * SEG
                                nc.tensor.matmul(
                                    ps[:, i * SEG:(i + 1) * SEG],
                                    lb[32 * i:32 * i + KR,
                                       t * 128:(t + 1) * 128],
                                    rb[32 * i:32 * i + KR, w:w + SEG],
                                    tile_position=(32 * i, 0),
                                )
                            if not do_act:
                                # tiny PSUM consumer so walrus can't DCE
                                # the matmuls in the mm-only probe
                                sc8 = jnk.tile([128, 8], jdt, tag="sc8")
                                nc.vector.tensor_scalar(
                                    out=sc8[:], in0=ps[:, 0:8], scalar1=1.0,
                                    scalar2=None, op0=mybir.AluOpType.mult,
                                    op1=mybir.AluOpType.max,
                                    accum_out=out_sb[:, 0:1],
                                )
                                continue
                            st = stg.tile([128, group], bf, tag="st")
                            nc.scalar.copy(st[:], ps[:])
                            gi = t * ncq + cq
                            csl = colmax[:, cq * group:(cq + 1) * group]
                            if t == 0 and do_row and do_col:
                                # fused: row accum-max + colmax init copy
                                nc.vector.tensor_scalar(
                                    out=csl, in0=st[:], scalar1=1.0,
                                    scalar2=None, op0=mybir.AluOpType.mult,
                                    op1=mybir.AluOpType.max,
                                    accum_out=rowpart[:, gi:gi + 1],
                                )
                                continue
                            if do_row:
                                sc = jnk.tile([128, group], jdt, tag="sc")
                                nc.vector.tensor_scalar(
                                    out=sc[:], in0=st[:], scalar1=1.0,
                                    scalar2=None, op0=mybir.AluOpType.mult,
                                    op1=mybir.AluOpType.max,
                                    accum_out=rowpart[:, gi:gi + 1],
                                )
                            if do_col:
                                if t == 0:
                                    nc.vector.tensor_scalar_mul(
                                        csl, st[:], 1.0
                                    )
                                else:
                                    nc.vector.tensor_tensor(
                                        out=csl, in0=st[:], in1=csl,
                                        op=mybir.AluOpType.max,
                                    )
                    # row tail: max over window-quads, sum over tiles
                    if do_row:
                        rmax = rowp.tile([128, nt], fp, tag="rmax")
                        if ncq > 1:
                            nc.vector.reduce_max(
                                rmax[:],
                                rowpart[:].rearrange(
                                    "p (t c) -> p t c", c=ncq
                                ),
                                axis=mybir.AxisListType.X,
                            )
                        else:
                            nc.vector.tensor_scalar_mul(
                                rmax[:], rowpart[:], 1.0
                            )
                        nc.vector.reduce_sum(
                            out_sb[:, 2 * b:2 * b + 1], rmax[:],
                            axis=mybir.AxisListType.X,
                        )
                    # col tail: reduce colmax over the partition axis
                    if do_col:
                        car = carp.tile([128, n], bf, tag="car")
                        nc.gpsimd.partition_all_reduce(
                            car[:], colmax[:], channels=128,
                            reduce_op=bass_isa.ReduceOp.max,
                        )
                        cjunk = rowp.tile([1, n], jdt, tag="cjunk")
                        nc.vector.tensor_scalar(
                            out=cjunk[:], in0=car[0:1, :], scalar1=1.0,
                            scalar2=None, op0=mybir.AluOpType.mult,
                            op1=mybir.AluOpType.add,
                            accum_out=out_sb[0:1, 2 * b + 1:2 * b + 2],
                        )

            body = emit_main
            if reps == 1:
                body()
            else:
                U = 4 if reps % 4 == 0 else 1
                with tc.For_i(0, reps // U, 1):
                    for _ in range(U):
                        body()

            nc.sync.dma_start(out=out_d[:], in_=out_sb[:])

    nc.compile()
    _built[key] = (nc, "Lblk", "Rblk", "out")
    return _built[key]


def _split(x):
    """fp32 array -> (hi bf16, lo bf16) with hi+lo == x to ~1e-5 rel."""
    hi = x.astype(BF16)
    lo = (x - hi.astype(np.float32)).astype(BF16)
    return hi, lo


def make_blocks(g, p):
    """g, p: [n, 3] fp32 -> (L, R) [13, n] bf16 matmul blocks.

    out[m, j] = sum_k L[k, m] * R[k, j]
              = gh.ph + gh.pl + gl.ph + (-|p|^2/2) + (-|g|^2/2)
              ~ g.p - |p|^2/2 - |g|^2/2 = -||g - p||^2 / 2
    """
    n = g.shape[0]
    gh, gl = _split(g)
    ph, pl = _split(p)
    gn = (-0.5 * np.square(g.astype(np.float64)).sum(-1)).astype(np.float32)
    pn = (-0.5 * np.square(p.astype(np.float64)).sum(-1)).astype(np.float32)
    gnh, gnl = _split(gn)
    pnh, pnl = _split(pn)
    ones = np.ones((1, n), dtype=BF16)

    L = np.empty((KR, n), dtype=BF16)
    L[0:3] = gh.T
    L[3:6] = gh.T
    L[6:9] = gl.T
    L[9] = ones
    L[10] = ones
    L[11] = gnh
    L[12] = gnl

    R = np.empty((KR, n), dtype=BF16)
    R[0:3] = ph.T
    R[3:6] = pl.T
    R[6:9] = ph.T
    R[9] = pnh
    R[10] = pnl
    R[11] = ones
    R[12] = ones
    return L, R


def pack_blocks(L, R, n=N_FULL):
    """[13, n] blocks -> [128, n] layouts replicated at partition
    offsets {0,32,64,96} for the 4-way row-group-packed matmuls."""
    L4 = np.zeros((128, n), dtype=BF16)
    R4 = np.zeros((128, n), dtype=BF16)
    for i in range(QUAD):
        L4[32 * i:32 * i + KR] = L
        R4[32 * i:32 * i + KR] = R
    return L4, R4


def shard_inputs(preds, gts, bloc=BLOC, ncores=NCORES):
    preds = np.asarray(preds, dtype=np.float32)
    gts = np.asarray(gts, dtype=np.float32)
    n = preds.shape[1]
    in_maps = []
    for c in range(ncores):
        Ls = np.empty((bloc, 128, n), dtype=BF16)
        Rs = np.empty((bloc, 128, n), dtype=BF16)
        for b in range(bloc):
            L, R = make_blocks(gts[c * bloc + b], preds[c * bloc + b])
            Ls[b], Rs[b] = pack_blocks(L, R, n)
        in_maps.append({"Lblk": Ls, "Rblk": Rs})
    return in_maps


def combine_outputs(outs, n=N_FULL, b=B):
    tot = np.sum([o.astype(np.float64).sum() for o in outs])
    return np.float32(-2.0 * tot / (b * n))


def kernel(preds, gts):
    nc, _, _, on = build()
    in_maps = shard_inputs(preds, gts)
    res = run_bass_kernel_spmd(nc, in_maps, core_ids=list(range(NCORES)))
    return combine_outputs([r[on] for r in res.results])


def _numpy_chamfer(preds, gts):
    tot = 0.0
    for b_ in range(preds.shape[0]):
        gg = (gts[b_] ** 2).sum(-1)
        pp = (preds[b_] ** 2).sum(-1)
        zz = gts[b_] @ preds[b_].T
        P = gg[:, None] + pp[None, :] - 2 * zz
        tot += P.min(axis=0).mean() + P.min(axis=1).mean()
    return tot / preds.shape[0]


if __name__ == "__main__":
    from concourse.bass_interp import CoreSim

    n = int(sys.argv[1]) if len(sys.argv) > 1 else 2048
    bloc = int(sys.argv[2]) if len(sys.argv) > 2 else 1
    reps = int(sys.argv[3]) if len(sys.argv) > 3 else 1
    nc, ln, rn, on = build(n=n, bloc=bloc, reps=reps)
    rng = np.random.default_rng(0)
    preds = rng.standard_normal((bloc, n, D)).astype(np.float32)
    gts = rng.standard_normal((bloc, n, D)).astype(np.float32)

    sim = CoreSim(nc)
    for bi in range(bloc):
        L, R = make_blocks(gts[bi], preds[bi])
        L4, R4 = pack_blocks(L, R, n)
        sim.tensor(ln)[bi] = L4
        sim.tensor(rn)[bi] = R4
    sim.simulate()
    got = combine_outputs([sim.tensor(on)], n=n, b=bloc)
    want = _numpy_chamfer(preds, gts)
    print("sim:", got, "numpy:", want, "rel err:", abs(got - want) / abs(want))


# revision 15
# speedup vs baseline: 1.2926x; 1.2926x over previous
"""Chamfer loss kernel v5 for Trainium2, 8 NeuronCores.

Math: T[i,j] = -||g_i - p_j||^2 / 2 computed in PSUM by K=13 bf16
matmuls using an exact hi/lo split (see make_blocks).

v5 = v2's DVE/ACT schedule + 4-way PE row-group packing:
  - K=13 fits in one 32-row group, so FOUR matmuls run concurrently at
    tile_position (32i, 0), i=0..3. Both operands are replicated at
    partition offsets {0,32,64,96} host-side.
  - Each PSUM group [128,2048] = ONE gt row-tile x FOUR adjacent
    512-wide pred windows (segment i = window 4*cq+i). This keeps the
    staged tile homogeneous in the row direction, so the v2-style
    big-FD DVE ops still apply:
      ACT: one [128,2048] PSUM->SBUF bf16 copy per group (~2us)
      row: one tensor_scalar accum-max FD=2048 (4x mode, fp8 junk out)
      col: one in-place tensor_tensor max FD=2048 into colmax
    First tile per window-quad uses v2's fused op (ts main-out writes
    colmax init + accum row max).
  - Col tail: gpsimd partition_all_reduce(max) + DVE accum-add.
  - Packed MMs cut PE busy from 214us to ~96us/core (hidden under DVE).

loss = -2 * (sum_i max_j T + sum_j max_i T) / (B*N), summed on host.

Sharding: batch dim 16 -> 2 per core.
"""

import sys

import numpy as np

sys.path.insert(0, "/opt/trn_rl_repo")

import ml_dtypes  # noqa: E402

import concourse.bass as bass  # noqa: E402
import concourse.mybir as mybir  # noqa: E402
import concourse.tile as tile  # noqa: E402
from concourse import bacc  # noqa: E402
from concourse.bass_utils import run_bass_kernel_spmd  # noqa: E402

BF16 = ml_dtypes.bfloat16

B, N_FULL, D = 16, 4096, 3
NCORES = 8
BLOC = B // NCORES  # batches per core
KR = 13  # matmul contraction rows
SEG = 512  # psum bank / matmul moving width
QUAD = 4  # windows packed per psum group (one per PE row group)

_built = {}


def build(n=N_FULL, bloc=BLOC, reps=1, probe="full", junk8=True, gcol=0):
    """Per-core Bass module. Returns (nc, "Lblk", "Rblk", "out")."""
    key = (n, bloc, reps, probe, junk8)
    if key in _built:
        return _built[key]
    do_act = probe != "mm"
    do_row = probe in ("full", "row")
    do_col = probe in ("full", "col")

    nt = n // 128  # row tiles (32)
    nc_ = n // SEG  # pred windows (8)
    ncq = nc_ // QUAD  # window quads (2)
    group = QUAD * SEG  # staged tile width (2048)

    nc = bacc.Bacc("TRN2", target_bir_lowering=False, debug=False)
    from concourse import bass_isa
    bf = mybir.dt.bfloat16
    fp = mybir.dt.float32
    jdt = mybir.dt.float8e4 if junk8 else bf

    L_d = nc.dram_tensor("Lblk", [bloc, KR, n], bf, kind="ExternalInput")
    R_d = nc.dram_tensor("Rblk", [bloc, KR, n], bf, kind="ExternalInput")
    out_d = nc.dram_tensor("out", [128, 2 * bloc], fp, kind="ExternalOutput")

    with tile.TileContext(nc) as tc:
        with (
            tc.tile_pool(name="blocks", bufs=1) as blocks,
            tc.tile_pool(name="stg", bufs=4) as stg,
            tc.tile_pool(name="jnk", bufs=2) as jnk,
            tc.tile_pool(name="colp", bufs=2) as colp,
            tc.tile_pool(name="carp", bufs=2) as carp,
            tc.tile_pool(name="rowp", bufs=2) as rowp,
            tc.tile_pool(name="small", bufs=1) as small,
            tc.tile_pool(name="psum", bufs=2, space="PSUM") as psum_pool,
        ):
            out_sb = small.tile([128, 2 * bloc], fp, tag="out_sb")
            nc.scalar.memzero(out_sb[:])

            lbs, rbs = [], []
            for b in range(bloc):
                lb = blocks.tile([128, n], bf, tag=f"lb{b}")
                rb = blocks.tile([128, n], bf, tag=f"rb{b}")
                # replicate the [13, n] blocks to partition offsets
                # {0,32,64,96} on-device (one-time, outside the rep loop)
                for i in range(QUAD):
                    nc.sync.dma_start(
                        out=lb[32 * i:32 * i + KR, :], in_=L_d[b])
                    nc.sync.dma_start(
                        out=rb[32 * i:32 * i + KR, :], in_=R_d[b])
                lbs.append(lb)
                rbs.append(rb)

            def emit_main():
                for b in range(bloc):
                    lb, rb = lbs[b], rbs[b]
                    colmax = colp.tile([128, n], bf, tag="colmax")
                    rowpart = rowp.tile([128, nt * ncq], fp, tag="rowpart")
                    for t in range(nt):
                        for cq in range(ncq):
                            ps = psum_pool.tile([128, group], fp, tag="ps")
                            for i in range(QUAD):
                                w = (QUAD * cq + i) * SEG
                                nc.tensor.matmul(
                                    ps[:, i * SEG:(i + 1) * SEG],
                                    lb[32 * i:32 * i + KR,
                                       t * 128:(t + 1) * 128],
                                    rb[32 * i:32 * i + KR, w:w + SEG],
                                    tile_position=(32 * i, 0),
                                )
                            if not do_act:
                                # tiny PSUM consumer so walrus can't DCE
                                # the matmuls in the mm-only probe
                                sc8 = jnk.tile([128, 8], jdt, tag="sc8")
                                nc.vector.tensor_scalar(
                                    out=sc8[:], in0=ps[:, 0:8], scalar1=1.0,
                                    scalar2=None, op0=mybir.AluOpType.mult,
                                    op1=mybir.AluOpType.max,
                                    accum_out=out_sb[:, 0:1],
                                )
                                continue
                            st = stg.tile([128, group], bf, tag="st")
                            nc.scalar.copy(st[:], ps[:])
                            gi = t * ncq + cq
                            csl = colmax[:, cq * group:(cq + 1) * group]
                            if t == 0 and do_row and do_col:
                                # fused: row accum-max + colmax init copy
                                nc.vector.tensor_scalar(
                                    out=csl, in0=st[:], scalar1=1.0,
                                    scalar2=None, op0=mybir.AluOpType.mult,
                                    op1=mybir.AluOpType.max,
                                    accum_out=rowpart[:, gi:gi + 1],
                                )
                                continue
                            if do_row:
                                sc = jnk.tile([128, group], jdt, tag="sc")
                                nc.vector.tensor_scalar(
                                    out=sc[:], in0=st[:], scalar1=1.0,
                                    scalar2=None, op0=mybir.AluOpType.mult,
                                    op1=mybir.AluOpType.max,
                                    accum_out=rowpart[:, gi:gi + 1],
                                )
                            if do_col:
                                if t == 0:
                                    nc.vector.tensor_scalar_mul(
                                        csl, st[:], 1.0
                                    )
                                else:
                                    nc.vector.tensor_tensor(
                                        out=csl, in0=st[:], in1=csl,
                                        op=mybir.AluOpType.max,
                                    )
                    # row tail: max over window-quads, sum over tiles
                    if do_row:
                        rmax = rowp.tile([128, nt], fp, tag="rmax")
                        if ncq > 1:
                            nc.vector.reduce_max(
                                rmax[:],
                                rowpart[:].rearrange(
                                    "p (t c) -> p t c", c=ncq
                                ),
                                axis=mybir.AxisListType.X,
                            )
                        else:
                            nc.vector.tensor_scalar_mul(
                                rmax[:], rowpart[:], 1.0
                            )
                        nc.vector.reduce_sum(
                            out_sb[:, 2 * b:2 * b + 1], rmax[:],
                            axis=mybir.AxisListType.X,
                        )
                    # col tail: reduce colmax over the partition axis
                    if do_col:
                        car = carp.tile([128, n], bf, tag="car")
                        nc.gpsimd.partition_all_reduce(
                            car[:], colmax[:], channels=128,
                            reduce_op=bass_isa.ReduceOp.max,
                        )
                        cjunk = rowp.tile([1, n], jdt, tag="cjunk")
                        nc.vector.tensor_scalar(
                            out=cjunk[:], in0=car[0:1, :], scalar1=1.0,
                            scalar2=None, op0=mybir.AluOpType.mult,
                            op1=mybir.AluOpType.add,
                            accum_out=out_sb[0:1, 2 * b + 1:2 * b + 2],
                        )

            body = emit_main
            if reps == 1:
                body()
            else:
                U = 4 if reps % 4 == 0 else 1
                with tc.For_i(0, reps // U, 1):
                    for _ in range(U):
                        body()

            nc.sync.dma_start(out=out_d[:], in_=out_sb[:])

    nc.compile()
    _built[key] = (nc, "Lblk", "Rblk", "out")
    return _built[key]


def _split(x):
    """fp32 array -> (hi bf16, lo bf16) with hi+lo == x to ~1e-5 rel."""
    hi = x.astype(BF16)
    lo = (x - hi.astype(np.float32)).astype(BF16)
    return hi, lo


def make_blocks(g, p):
    """g, p: [n, 3] fp32 -> (L, R) [13, n] bf16 matmul blocks.

    out[m, j] = sum_k L[k, m] * R[k, j]
              = gh.ph + gh.pl + gl.ph + (-|p|^2/2) + (-|g|^2/2)
              ~ g.p - |p|^2/2 - |g|^2/2 = -||g - p||^2 / 2
    """
    n = g.shape[0]
    gh, gl = _split(g)
    ph, pl = _split(p)
    gn = (-0.5 * np.square(g.astype(np.float64)).sum(-1)).astype(np.float32)
    pn = (-0.5 * np.square(p.astype(np.float64)).sum(-1)).astype(np.float32)
    gnh, gnl = _split(gn)
    pnh, pnl = _split(pn)
    ones = np.ones((1, n), dtype=BF16)

    L = np.empty((KR, n), dtype=BF16)
    L[0:3] = gh.T
    L[3:6] = gh.T
    L[6:9] = gl.T
    L[9] = ones
    L[10] = ones
    L[11] = gnh
    L[12] = gnl

    R = np.empty((KR, n), dtype=BF16)
    R[0:3] = ph.T
    R[3:6] = pl.T
    R[6:9] = ph.T
    R[9] = pnh
    R[10] = pnl
    R[11] = ones
    R[12] = ones
    return L, R


def shard_inputs(preds, gts, bloc=BLOC, ncores=NCORES):
    preds = np.asarray(preds, dtype=np.float32)
    gts = np.asarray(gts, dtype=np.float32)
    n = preds.shape[1]
    in_maps = []
    for c in range(ncores):
        Ls = np.empty((bloc, KR, n), dtype=BF16)
        Rs = np.empty((bloc, KR, n), dtype=BF16)
        for b in range(bloc):
            Ls[b], Rs[b] = make_blocks(gts[c * bloc + b],
                                       preds[c * bloc + b])
        in_maps.append({"Lblk": Ls, "Rblk": Rs})
    return in_maps


def combine_outputs(outs, n=N_FULL, b=B):
    tot = np.sum([o.astype(np.float64).sum() for o in outs])
    return np.float32(-2.0 * tot / (b * n))


def kernel(preds, gts):
    nc, _, _, on = build()
    in_maps = shard_inputs(preds, gts)
    res = run_bass_kernel_spmd(nc, in_maps, core_ids=list(range(NCORES)))
    return combine_outputs([r[on] for r in res.results])


def _numpy_chamfer(preds, gts):
    tot = 0.0
    for b_ in range(preds.shape[0]):
        gg = (gts[b_] ** 2).sum(-1)
        pp = (preds[b_] ** 2).sum(-1)
        zz = gts[b_] @ preds[b_].T
        P = gg[:, None] + pp[None, :] - 2 * zz
        tot += P.min(axis=0).mean() + P.min(axis=1).mean()
    return tot / preds.shape[0]


if __name__ == "__main__":
    from concourse.bass_interp import CoreSim

    n = int(sys.argv[1]) if len(sys.argv) > 1 else 2048
    bloc = int(sys.argv[2]) if len(sys.argv) > 2 else 1
    reps = int(sys.argv[3]) if len(sys.argv) > 3 else 1
    nc, ln, rn, on = build(n=n, bloc=bloc, reps=reps)
    rng = np.random.default_rng(0)
    preds = rng.standard_normal((bloc, n, D)).astype(np.float32)
    gts = rng.standard_normal((bloc, n, D)).astype(np.float32)

    sim = CoreSim(nc)
    for bi in range(bloc):
        L, R = make_blocks(gts[bi], preds[bi])
        sim.tensor(ln)[bi] = L
        sim.tensor(rn)[bi] = R
    sim.simulate()
    got = combine_outputs([sim.tensor(on)], n=n, b=bloc)
    want = _numpy_chamfer(preds, gts)
    print("sim:", got, "numpy:", want, "rel err:", abs(got - want) / abs(want))
